# revision 17
# baseline (speedup 1.0000x reference)
"""Trainium2 Bass kernel for nn_CanonicalCov1D (strided dual-projection covariance).

Math (reference):
  shift = W = 128, STRIDE = 8, L = T - 128 = 8064, NWIN = 993
  win1[b,n,:] = X[b, 8n : 8n+128],  win2[b,n,:] = X[b, 128+8n : 256+8n]
  proj_i = win_i @ weight_i  (per (LAT, C))
  cov[b,n,c] = mean_l[(proj1 - mean_l proj1) * (proj2 - mean_l proj2)] + bias

Key simplifications:
  * Centering projections over LAT == projecting with LAT-centered weights:
    center (and 1/LAT-scale) weights on the host, skip mean subtraction.
  * win2[n] == win_full[n+16] (shift = 16*STRIDE): one im2col serves both.
  * l-major weight layout ([w, l*64+c]) puts (l, c) on PSUM partitions, so
    the LAT-reduction becomes a constant-selector matmul that ACCUMULATES
    across all 16 chunks in one PSUM bank — no DVE reduction pass at all.
  * Output lands [c(64 partitions), n] so the bias folds into the scalar
    engine's PSUM->SBUF exit copy (per-partition bias). Final [b, c, n] ->
    [b, n, c] permute happens on the host during unsharding.

Per-core device pipeline (data-parallel over batch, 4 batches/core):
  1. dma_start_transpose builds winT [128(w), 1040(n)] bf16 from the
     overlapping-window view of X (xbar transpose).
  2. per (batch, 512-window tile t, chunk j of 16):
       p1 = W1c_j^T @ winT[:, t]        [128(2l x 64c), 512] PSUM
       p2 = W2c_j^T @ winT[:, t + 16]
       ACT: p1 -> SBUF; DVE: p12 = p1c * p2 (bf16)
       PE:  selout += sel^T @ p12       [64(c), 512] PSUM, accumulate over j
  3. ACT: selout + bias -> SBUF, DMA out as [b, c, n].
"""

import numpy as np

# ---- problem constants (hardcoded; kernel.py must be self-contained) ----
B, T = 32, 8192
W, LAT, C = 128, 32, 64
STRIDE = 8
NWIN = 993            # output windows
NWINF = 1009          # windows incl. +16 shift for proj2
NPAD = 1040           # winT free size (2*512 + 16)
N_CORES = 8
BPC = B // N_CORES    # batches per core
NTILES = 2            # 512-window tiles (512 + 481)
NCHUNKS = 16          # weight-column chunks of 128 (= 2 LAT values each)

_CACHE = {}


def _build():
    """Build the per-core Bass program."""
    import concourse.bass as bass
    import concourse.mybir as mybir
    import concourse.tile as tile
    from concourse import bacc

    f32 = mybir.dt.float32
    bf16 = mybir.dt.bfloat16

    nc = bacc.Bacc(
        "TRN2",
        target_bir_lowering=False,
        debug=False,
        enable_asserts=False,
    )

    x_dram = nc.dram_tensor("x", [BPC, T + 256], bf16, kind="ExternalInput")
    w_dram = nc.dram_tensor("w", [W, 2 * LAT * C], bf16, kind="ExternalInput")
    sel_dram = nc.dram_tensor("sel", [W, C], bf16, kind="ExternalInput")
    bias_dram = nc.dram_tensor("bias", [C, 1], f32, kind="ExternalInput")
    out_dram = nc.dram_tensor("out", [BPC, C, NWIN], f32, kind="ExternalOutput")

    with tile.TileContext(nc) as tc:
        with (
            tc.tile_pool(name="consts", bufs=1) as consts,
            tc.tile_pool(name="wins", bufs=2) as wins,
            tc.tile_pool(name="prods", bufs=4) as prods,
            tc.tile_pool(name="outs", bufs=2) as outs,
            tc.tile_pool(name="psum", bufs=1, space="PSUM") as psum,
        ):
            unit_idx = 0
            # Tile serializes every xbar-mode transition (transpose vs copy
            # DMA), so order matters: first the one weight quarter + sel the
            # first matmuls need, then ALL im2col transposes back-to-back,
            # then the remaining const loads.
            w_sb = consts.tile([W, 2 * LAT * C], bf16)
            nc.sync.dma_start(w_sb[:, 0:1024], w_dram.ap()[:, 0:1024])
            sel_sb = consts.tile([W, C], bf16)
            nc.sync.dma_start(sel_sb[:], sel_dram.ap())
            winTs = []
            for b in range(BPC):
                wt = wins.tile([128, NPAD], bf16, name=f"winT{b}", tag="winT", bufs=4)
                v_main = bass.AP(
                    tensor=x_dram,
                    offset=b * (T + 256),
                    ap=[[STRIDE, NPAD], [1, W]],
                )
                nc.sync.dma_start_transpose(wt[:], v_main)
                winTs.append(wt)
            for wq in range(1, 4):
                nc.sync.dma_start(
                    w_sb[:, wq * 1024 : wq * 1024 + 1024],
                    w_dram.ap()[:, wq * 1024 : wq * 1024 + 1024],
                )
            bias_sb = consts.tile([C, 1], f32)
            nc.sync.dma_start(bias_sb[:], bias_dram.ap())

            for b in range(BPC):
                winT = winTs[b]

                for t in range(NTILES):
                    selout = psum.tile([C, 512], f32, tag="selout", bufs=1)
                    rhs1 = winT[:, t * 512 : t * 512 + 512]
                    rhs2 = winT[:, t * 512 + 16 : t * 512 + 528]
                    for jp in range(NCHUNKS // 2):
                        p12s = []
                        for j in (2 * jp, 2 * jp + 1):
                            p1 = psum.tile([128, 512], f32, tag="p1", bufs=4)
                            p2 = psum.tile([128, 512], f32, tag="p2", bufs=3)
                            nc.tensor.matmul(
                                p1[:],
                                w_sb[:, j * 128 : j * 128 + 128],
                                rhs1,
                                start=True,
                                stop=True,
                            )
                            nc.tensor.matmul(
                                p2[:],
                                w_sb[:, 2048 + j * 128 : 2048 + j * 128 + 128],
                                rhs2,
                                start=True,
                                stop=True,
                            )
                            # only one PSUM operand per DVE op: stage p1
                            # through SBUF on the scalar engine
                            p1c = prods.tile([128, 512], f32, tag="p1c", bufs=5)
                            nc.scalar.copy(p1c[:], p1[:])
                            p12 = prods.tile([128, 512], bf16, tag="p12", bufs=6)
                            nc.vector.tensor_mul(p12[:], p1c[:], p2[:])
                            p12s.append(p12)
                        # pre-add the chunk pair on the (otherwise idle)
                        # GPSIMD engine, halving the selector matmuls
                        p12sum = prods.tile([128, 512], bf16, tag="p12sum", bufs=4)
                        nc.gpsimd.tensor_add(p12sum[:], p12s[0][:], p12s[1][:])
                        # LAT-reduction on the tensor engine: constant
                        # selector sums l-rows per c, accumulating in PSUM
                        nc.tensor.matmul(
                            selout[:],
                            sel_sb[:],
                            p12sum[:],
                            start=(jp == 0),
                            stop=(jp == NCHUNKS // 2 - 1),
                        )
                    # exit + bias in one scalar-engine op
                    st = outs.tile([C, 512], f32, tag="st")
                    nc.scalar.activation(
                        st[:],
                        selout[:],
                        mybir.ActivationFunctionType.Identity,
                        bias=bias_sb[:],
                    )
                    n0 = t * 512
                    n1 = min(NWIN, n0 + 512)
                    nc.sync.dma_start(
                        out_dram.ap()[b, :, n0:n1], st[:, 0 : n1 - n0]
                    )

    nc.compile()
    return nc


def _prep_inputs(X, weight1, weight2, bias):
    import ml_dtypes

    X = np.asarray(X, dtype=np.float32)
    weight1 = np.asarray(weight1, dtype=np.float32)
    weight2 = np.asarray(weight2, dtype=np.float32)
    bias = np.asarray(bias, dtype=np.float32)

    # center over LAT, fold 1/LAT into proj1's weights; l-major layout
    w1c = weight1 - weight1.mean(axis=1, keepdims=True)
    w2c = weight2 - weight2.mean(axis=1, keepdims=True)
    w1p = (w1c / LAT).reshape(W, LAT * C)
    w2p = w2c.reshape(W, LAT * C)
    wcat = np.concatenate([w1p, w2p], axis=1).astype(ml_dtypes.bfloat16)

    xpad = np.zeros((B, T + 256), dtype=np.float32)
    xpad[:, :T] = X
    xb = xpad.astype(ml_dtypes.bfloat16)
    sel = (np.arange(W)[:, None] % C == np.arange(C)[None, :]).astype(
        ml_dtypes.bfloat16
    )
    bias_col = np.ascontiguousarray(bias[:, None]).astype(np.float32)

    in_maps = []
    for i in range(N_CORES):
        in_maps.append(
            {
                "x": np.ascontiguousarray(xb[i * BPC : (i + 1) * BPC]),
                "w": wcat,
                "sel": sel,
                "bias": bias_col,
            }
        )
    return in_maps


def run_with_results(X, weight1, weight2, bias, trace=False, trace_cores=None):
    from concourse import bass_utils

    if "nc" not in _CACHE:
        _CACHE["nc"] = _build()
    nc = _CACHE["nc"]
    in_maps = _prep_inputs(X, weight1, weight2, bias)
    res = bass_utils.run_bass_kernel_spmd(
        nc,
        in_maps,
        core_ids=list(range(N_CORES)),
        trace=trace,
        trace_cores=trace_cores,
    )
    # results are [b, c, n]; transpose to [b, n, c] while unsharding
    out = np.concatenate(
        [res.results[i]["out"] for i in range(N_CORES)], axis=0
    ).transpose(0, 2, 1)
    return np.ascontiguousarray(out, dtype=np.float32), res


def kernel(**inputs):
    out, _ = run_with_results(
        inputs["X"], inputs["weight1"], inputs["weight2"], inputs["bias"]
    )
    return out


# revision 18
# speedup vs baseline: 1.0092x; 1.0092x over previous
"""Trainium2 Bass kernel for nn_CanonicalCov1D (strided dual-projection covariance).

Math (reference):
  shift = W = 128, STRIDE = 8, L = T - 128 = 8064, NWIN = 993
  win1[b,n,:] = X[b, 8n : 8n+128],  win2[b,n,:] = X[b, 128+8n : 256+8n]
  proj_i = win_i @ weight_i  (per (LAT, C))
  cov[b,n,c] = mean_l[(proj1 - mean_l proj1) * (proj2 - mean_l proj2)] + bias

Key simplifications:
  * Centering projections over LAT == projecting with LAT-centered weights:
    center (and 1/LAT-scale) weights on the host, skip mean subtraction.
  * win2[n] == win_full[n+16] (shift = 16*STRIDE): one im2col serves both.
  * l-major weight layout ([w, l*64+c]) puts (l, c) on PSUM partitions, so
    the LAT-reduction becomes a constant-selector matmul that ACCUMULATES
    across all 16 chunks in one PSUM bank — no DVE reduction pass at all.
  * Output lands [c(64 partitions), n] so the bias folds into the scalar
    engine's PSUM->SBUF exit copy (per-partition bias). Final [b, c, n] ->
    [b, n, c] permute happens on the host during unsharding.

Per-core device pipeline (data-parallel over batch, 4 batches/core):
  1. dma_start_transpose builds winT [128(w), 1040(n)] bf16 from the
     overlapping-window view of X (xbar transpose).
  2. per (batch, 512-window tile t, chunk j of 16):
       p1 = W1c_j^T @ winT[:, t]        [128(2l x 64c), 512] PSUM
       p2 = W2c_j^T @ winT[:, t + 16]
       ACT: p1 -> SBUF; DVE: p12 = p1c * p2 (bf16)
       PE:  selout += sel^T @ p12       [64(c), 512] PSUM, accumulate over j
  3. ACT: selout + bias -> SBUF, DMA out as [b, c, n].
"""

import numpy as np

# ---- problem constants (hardcoded; kernel.py must be self-contained) ----
B, T = 32, 8192
W, LAT, C = 128, 32, 64
STRIDE = 8
NWIN = 993            # output windows
NWINF = 1009          # windows incl. +16 shift for proj2
NPAD = 1040           # winT free size (2*512 + 16)
N_CORES = 8
BPC = B // N_CORES    # batches per core
NTILES = 2            # 512-window tiles (512 + 481)
NCHUNKS = 16          # weight-column chunks of 128 (= 2 LAT values each)

_CACHE = {}


def _build():
    """Build the per-core Bass program."""
    import concourse.bass as bass
    import concourse.mybir as mybir
    import concourse.tile as tile
    from concourse import bacc

    f32 = mybir.dt.float32
    bf16 = mybir.dt.bfloat16

    nc = bacc.Bacc(
        "TRN2",
        target_bir_lowering=False,
        debug=False,
        enable_asserts=False,
    )

    x_dram = nc.dram_tensor("x", [BPC, T + 256], bf16, kind="ExternalInput")
    w_dram = nc.dram_tensor("w", [W, 2 * LAT * C], bf16, kind="ExternalInput")
    sel_dram = nc.dram_tensor("sel", [W, C], bf16, kind="ExternalInput")
    bias_dram = nc.dram_tensor("bias", [C, 1], f32, kind="ExternalInput")
    out_dram = nc.dram_tensor("out", [BPC, C, NWIN], f32, kind="ExternalOutput")

    with tile.TileContext(nc) as tc:
        with (
            tc.tile_pool(name="consts", bufs=1) as consts,
            tc.tile_pool(name="wins", bufs=2) as wins,
            tc.tile_pool(name="prods", bufs=4) as prods,
            tc.tile_pool(name="outs", bufs=2) as outs,
            tc.tile_pool(name="psum", bufs=1, space="PSUM") as psum,
        ):
            unit_idx = 0
            # Tile serializes every xbar-mode transition (transpose vs copy
            # DMA), so order matters: first the one weight quarter + sel the
            # first matmuls need, then ALL im2col transposes back-to-back,
            # then the remaining const loads.
            w_sb = consts.tile([W, 2 * LAT * C], bf16)
            # quarters 0 (proj1 j<8) and 2 (proj2 j<8) feed the first chunks
            nc.sync.dma_start(w_sb[:, 0:1024], w_dram.ap()[:, 0:1024])
            nc.sync.dma_start(w_sb[:, 2048:3072], w_dram.ap()[:, 2048:3072])
            winTs = []
            for b in range(BPC):
                wt = wins.tile([128, NPAD], bf16, name=f"winT{b}", tag="winT", bufs=4)
                v_main = bass.AP(
                    tensor=x_dram,
                    offset=b * (T + 256),
                    ap=[[STRIDE, NPAD], [1, W]],
                )
                nc.sync.dma_start_transpose(wt[:], v_main)
                winTs.append(wt)
            for wq in (1, 3):
                nc.sync.dma_start(
                    w_sb[:, wq * 1024 : wq * 1024 + 1024],
                    w_dram.ap()[:, wq * 1024 : wq * 1024 + 1024],
                )
            sel_sb = consts.tile([W, C], bf16)
            nc.sync.dma_start(sel_sb[:], sel_dram.ap())
            bias_sb = consts.tile([C, 1], f32)
            nc.sync.dma_start(bias_sb[:], bias_dram.ap())

            for b in range(BPC):
                winT = winTs[b]

                for t in range(NTILES):
                    selout = psum.tile([C, 512], f32, tag="selout", bufs=1)
                    rhs1 = winT[:, t * 512 : t * 512 + 512]
                    rhs2 = winT[:, t * 512 + 16 : t * 512 + 528]
                    for jp in range(NCHUNKS // 2):
                        p12s = []
                        for j in (2 * jp, 2 * jp + 1):
                            p1 = psum.tile([128, 512], f32, tag="p1", bufs=4)
                            p2 = psum.tile([128, 512], f32, tag="p2", bufs=3)
                            nc.tensor.matmul(
                                p1[:],
                                w_sb[:, j * 128 : j * 128 + 128],
                                rhs1,
                                start=True,
                                stop=True,
                            )
                            nc.tensor.matmul(
                                p2[:],
                                w_sb[:, 2048 + j * 128 : 2048 + j * 128 + 128],
                                rhs2,
                                start=True,
                                stop=True,
                            )
                            # only one PSUM operand per DVE op: stage p1
                            # through SBUF on the scalar engine
                            p1c = prods.tile([128, 512], f32, tag="p1c", bufs=5)
                            nc.scalar.copy(p1c[:], p1[:])
                            p12 = prods.tile([128, 512], bf16, tag="p12", bufs=6)
                            nc.vector.tensor_mul(p12[:], p1c[:], p2[:])
                            p12s.append(p12)
                        # pre-add the chunk pair on the (otherwise idle)
                        # GPSIMD engine, halving the selector matmuls
                        p12sum = prods.tile([128, 512], bf16, tag="p12sum", bufs=4)
                        nc.gpsimd.tensor_add(p12sum[:], p12s[0][:], p12s[1][:])
                        # LAT-reduction on the tensor engine: constant
                        # selector sums l-rows per c, accumulating in PSUM
                        nc.tensor.matmul(
                            selout[:],
                            sel_sb[:],
                            p12sum[:],
                            start=(jp == 0),
                            stop=(jp == NCHUNKS // 2 - 1),
                        )
                    # exit + bias in one scalar-engine op
                    st = outs.tile([C, 512], f32, tag="st")
                    nc.scalar.activation(
                        st[:],
                        selout[:],
                        mybir.ActivationFunctionType.Identity,
                        bias=bias_sb[:],
                    )
                    n0 = t * 512
                    n1 = min(NWIN, n0 + 512)
                    nc.sync.dma_start(
                        out_dram.ap()[b, :, n0:n1], st[:, 0 : n1 - n0]
                    )

    nc.compile()
    return nc


def _prep_inputs(X, weight1, weight2, bias):
    import ml_dtypes

    X = np.asarray(X, dtype=np.float32)
    weight1 = np.asarray(weight1, dtype=np.float32)
    weight2 = np.asarray(weight2, dtype=np.float32)
    bias = np.asarray(bias, dtype=np.float32)

    # center over LAT, fold 1/LAT into proj1's weights; l-major layout
    w1c = weight1 - weight1.mean(axis=1, keepdims=True)
    w2c = weight2 - weight2.mean(axis=1, keepdims=True)
    w1p = (w1c / LAT).reshape(W, LAT * C)
    w2p = w2c.reshape(W, LAT * C)
    wcat = np.concatenate([w1p, w2p], axis=1).astype(ml_dtypes.bfloat16)

    xpad = np.zeros((B, T + 256), dtype=np.float32)
    xpad[:, :T] = X
    xb = xpad.astype(ml_dtypes.bfloat16)
    sel = (np.arange(W)[:, None] % C == np.arange(C)[None, :]).astype(
        ml_dtypes.bfloat16
    )
    bias_col = np.ascontiguousarray(bias[:, None]).astype(np.float32)

    in_maps = []
    for i in range(N_CORES):
        in_maps.append(
            {
                "x": np.ascontiguousarray(xb[i * BPC : (i + 1) * BPC]),
                "w": wcat,
                "sel": sel,
                "bias": bias_col,
            }
        )
    return in_maps


def run_with_results(X, weight1, weight2, bias, trace=False, trace_cores=None):
    from concourse import bass_utils

    if "nc" not in _CACHE:
        _CACHE["nc"] = _build()
    nc = _CACHE["nc"]
    in_maps = _prep_inputs(X, weight1, weight2, bias)
    res = bass_utils.run_bass_kernel_spmd(
        nc,
        in_maps,
        core_ids=list(range(N_CORES)),
        trace=trace,
        trace_cores=trace_cores,
    )
    # results are [b, c, n]; transpose to [b, n, c] while unsharding
    out = np.concatenate(
        [res.results[i]["out"] for i in range(N_CORES)], axis=0
    ).transpose(0, 2, 1)
    return np.ascontiguousarray(out, dtype=np.float32), res


def kernel(**inputs):
    out, _ = run_with_results(
        inputs["X"], inputs["weight1"], inputs["weight2"], inputs["bias"]
    )
    return out


# revision 19
# speedup vs baseline: 1.0232x; 1.0139x over previous
"""Trainium2 Bass kernel for nn_CanonicalCov1D (strided dual-projection covariance).

Math (reference):
  shift = W = 128, STRIDE = 8, L = T - 128 = 8064, NWIN = 993
  win1[b,n,:] = X[b, 8n : 8n+128],  win2[b,n,:] = X[b, 128+8n : 256+8n]
  proj_i = win_i @ weight_i  (per (LAT, C))
  cov[b,n,c] = mean_l[(proj1 - mean_l proj1) * (proj2 - mean_l proj2)] + bias

Key simplifications:
  * Centering projections over LAT == projecting with LAT-centered weights:
    center (and 1/LAT-scale) weights on the host, skip mean subtraction.
  * win2[n] == win_full[n+16] (shift = 16*STRIDE): one im2col serves both.
  * l-major weight layout ([w, l*64+c]) puts (l, c) on PSUM partitions, so
    the LAT-reduction becomes a constant-selector matmul that ACCUMULATES
    across all 16 chunks in one PSUM bank — no DVE reduction pass at all.
  * Output lands [c(64 partitions), n] so the bias folds into the scalar
    engine's PSUM->SBUF exit copy (per-partition bias). Final [b, c, n] ->
    [b, n, c] permute happens on the host during unsharding.

Per-core device pipeline (data-parallel over batch, 4 batches/core):
  1. dma_start_transpose builds winT [128(w), 1040(n)] bf16 from the
     overlapping-window view of X (xbar transpose).
  2. per (batch, 512-window tile t, chunk j of 16):
       p1 = W1c_j^T @ winT[:, t]        [128(2l x 64c), 512] PSUM
       p2 = W2c_j^T @ winT[:, t + 16]
       ACT: p1 -> SBUF; DVE: p12 = p1c * p2 (bf16)
       PE:  selout += sel^T @ p12       [64(c), 512] PSUM, accumulate over j
  3. ACT: selout + bias -> SBUF, DMA out as [b, c, n].
"""

import numpy as np

# ---- problem constants (hardcoded; kernel.py must be self-contained) ----
B, T = 32, 8192
W, LAT, C = 128, 32, 64
STRIDE = 8
NWIN = 993            # output windows
NWINF = 1009          # windows incl. +16 shift for proj2
NPAD = 1040           # winT free size (2*512 + 16)
N_CORES = 8
BPC = B // N_CORES    # batches per core
NTILES = 2            # 512-window tiles (512 + 481)
NCHUNKS = 16          # weight-column chunks of 128 (= 2 LAT values each)

_CACHE = {}


def _build():
    """Build the per-core Bass program."""
    import concourse.bass as bass
    import concourse.mybir as mybir
    import concourse.tile as tile
    from concourse import bacc

    f32 = mybir.dt.float32
    bf16 = mybir.dt.bfloat16

    nc = bacc.Bacc(
        "TRN2",
        target_bir_lowering=False,
        debug=False,
        enable_asserts=False,
    )

    x_dram = nc.dram_tensor("x", [BPC, T + 256], bf16, kind="ExternalInput")
    w_dram = nc.dram_tensor("w", [W, 2 * LAT * C], bf16, kind="ExternalInput")
    sel_dram = nc.dram_tensor("sel", [W, C], bf16, kind="ExternalInput")
    bias_dram = nc.dram_tensor("bias", [C, 1], f32, kind="ExternalInput")
    out_dram = nc.dram_tensor("out", [BPC, C, NWIN], f32, kind="ExternalOutput")

    with tile.TileContext(nc) as tc:
        with (
            tc.tile_pool(name="consts", bufs=1) as consts,
            tc.tile_pool(name="wins", bufs=2) as wins,
            tc.tile_pool(name="prods", bufs=4) as prods,
            tc.tile_pool(name="outs", bufs=2) as outs,
            tc.tile_pool(name="psum", bufs=1, space="PSUM") as psum,
        ):
            unit_idx = 0
            # Tile serializes every xbar-mode transition (transpose vs copy
            # DMA), so order matters: first the one weight quarter + sel the
            # first matmuls need, then ALL im2col transposes back-to-back,
            # then the remaining const loads.
            w_sb = consts.tile([W, 2 * LAT * C], bf16)
            # quarters 0 (proj1 j<8) and 2 (proj2 j<8) feed the first chunks
            nc.sync.dma_start(w_sb[:, 0:1024], w_dram.ap()[:, 0:1024])
            nc.sync.dma_start(w_sb[:, 2048:3072], w_dram.ap()[:, 2048:3072])
            winTs = []
            for b in range(BPC):
                wt = wins.tile([128, NPAD], bf16, name=f"winT{b}", tag="winT", bufs=4)
                v_main = bass.AP(
                    tensor=x_dram,
                    offset=b * (T + 256),
                    ap=[[STRIDE, NPAD], [1, W]],
                )
                nc.sync.dma_start_transpose(wt[:], v_main)
                winTs.append(wt)
            for wq in (1, 3):
                nc.sync.dma_start(
                    w_sb[:, wq * 1024 : wq * 1024 + 1024],
                    w_dram.ap()[:, wq * 1024 : wq * 1024 + 1024],
                )
            sel_sb = consts.tile([W, C], bf16)
            nc.sync.dma_start(sel_sb[:], sel_dram.ap())
            bias_sb = consts.tile([C, 1], f32)
            nc.sync.dma_start(bias_sb[:], bias_dram.ap())

            for b in range(BPC):
                winT = winTs[b]

                for t in range(NTILES):
                    selout = psum.tile([C, 512], f32, tag="selout", bufs=1)
                    rhs1 = winT[:, t * 512 : t * 512 + 512]
                    rhs2 = winT[:, t * 512 + 16 : t * 512 + 528]
                    for jp in range(NCHUNKS // 2):
                        # both chunks' p1 share one 2-bank PSUM tile so a
                        # single scalar-engine op exits the pair to SBUF
                        p1pair = psum.tile([128, 1024], f32, tag="p1pair", bufs=2)
                        p2s = []
                        for qi, j in enumerate((2 * jp, 2 * jp + 1)):
                            nc.tensor.matmul(
                                p1pair[:, qi * 512 : qi * 512 + 512],
                                w_sb[:, j * 128 : j * 128 + 128],
                                rhs1,
                                start=True,
                                stop=True,
                            )
                            p2 = psum.tile([128, 512], f32, tag="p2", bufs=3)
                            nc.tensor.matmul(
                                p2[:],
                                w_sb[:, 2048 + j * 128 : 2048 + j * 128 + 128],
                                rhs2,
                                start=True,
                                stop=True,
                            )
                            p2s.append(p2)
                        # only one PSUM operand per DVE op: stage p1 pair
                        # through SBUF on the scalar engine
                        p1c = prods.tile([128, 1024], f32, tag="p1c", bufs=4)
                        nc.scalar.copy(p1c[:], p1pair[:])
                        p12s = []
                        for qi in range(2):
                            p12 = prods.tile([128, 512], bf16, tag="p12", bufs=8)
                            nc.vector.tensor_mul(
                                p12[:], p1c[:, qi * 512 : qi * 512 + 512], p2s[qi][:]
                            )
                            p12s.append(p12)
                        # pre-add the chunk pair on the (otherwise idle)
                        # GPSIMD engine, halving the selector matmuls
                        p12sum = prods.tile([128, 512], bf16, tag="p12sum", bufs=4)
                        nc.gpsimd.tensor_add(p12sum[:], p12s[0][:], p12s[1][:])
                        # LAT-reduction on the tensor engine: constant
                        # selector sums l-rows per c, accumulating in PSUM
                        nc.tensor.matmul(
                            selout[:],
                            sel_sb[:],
                            p12sum[:],
                            start=(jp == 0),
                            stop=(jp == NCHUNKS // 2 - 1),
                        )
                    # exit + bias in one scalar-engine op
                    st = outs.tile([C, 512], f32, tag="st")
                    nc.scalar.activation(
                        st[:],
                        selout[:],
                        mybir.ActivationFunctionType.Identity,
                        bias=bias_sb[:],
                    )
                    n0 = t * 512
                    n1 = min(NWIN, n0 + 512)
                    nc.sync.dma_start(
                        out_dram.ap()[b, :, n0:n1], st[:, 0 : n1 - n0]
                    )

    nc.compile()
    return nc


def _prep_inputs(X, weight1, weight2, bias):
    import ml_dtypes

    X = np.asarray(X, dtype=np.float32)
    weight1 = np.asarray(weight1, dtype=np.float32)
    weight2 = np.asarray(weight2, dtype=np.float32)
    bias = np.asarray(bias, dtype=np.float32)

    # center over LAT, fold 1/LAT into proj1's weights; l-major layout
    w1c = weight1 - weight1.mean(axis=1, keepdims=True)
    w2c = weight2 - weight2.mean(axis=1, keepdims=True)
    w1p = (w1c / LAT).reshape(W, LAT * C)
    w2p = w2c.reshape(W, LAT * C)
    wcat = np.concatenate([w1p, w2p], axis=1).astype(ml_dtypes.bfloat16)

    xpad = np.zeros((B, T + 256), dtype=np.float32)
    xpad[:, :T] = X
    xb = xpad.astype(ml_dtypes.bfloat16)
    sel = (np.arange(W)[:, None] % C == np.arange(C)[None, :]).astype(
        ml_dtypes.bfloat16
    )
    bias_col = np.ascontiguousarray(bias[:, None]).astype(np.float32)

    in_maps = []
    for i in range(N_CORES):
        in_maps.append(
            {
                "x": np.ascontiguousarray(xb[i * BPC : (i + 1) * BPC]),
                "w": wcat,
                "sel": sel,
                "bias": bias_col,
            }
        )
    return in_maps


def run_with_results(X, weight1, weight2, bias, trace=False, trace_cores=None):
    from concourse import bass_utils

    if "nc" not in _CACHE:
        _CACHE["nc"] = _build()
    nc = _CACHE["nc"]
    in_maps = _prep_inputs(X, weight1, weight2, bias)
    res = bass_utils.run_bass_kernel_spmd(
        nc,
        in_maps,
        core_ids=list(range(N_CORES)),
        trace=trace,
        trace_cores=trace_cores,
    )
    # results are [b, c, n]; transpose to [b, n, c] while unsharding
    out = np.concatenate(
        [res.results[i]["out"] for i in range(N_CORES)], axis=0
    ).transpose(0, 2, 1)
    return np.ascontiguousarray(out, dtype=np.float32), res


def kernel(**inputs):
    out, _ = run_with_results(
        inputs["X"], inputs["weight1"], inputs["weight2"], inputs["bias"]
    )
    return out
